# revision 22
# baseline (speedup 1.0000x reference)
"""Kronecker layer forward on 8 TRN2 NeuronCores.

Computes y = gelu_exact(x @ kron(B, A)) + bias for
  x [16384, 4096] f32, A [64, 64], B [64, 64], bias [4096].

Math: with x3 = x.reshape(n, 64, 64) (feature f = i*64 + k),
  y[b, j*64+l] = sum_{i,k} x3[b,i,k] * B[i,j] * A[k,l].

Per supertile s we pick 4 tokens t(g,h) = g*(tpc/2) + h*(tpc/4) + s
(g,h in {0,1}) and form one 128x128 SBUF tile
  xt[(g,i), (h,k)] = x[t(g,h), i*64+k]
then chain two TensorE matmuls in fp16:

  MM1 (data-stationary): o1 = xt.T @ blockdiag(B,B)
      -> o1[(h,k), (g,j)] = sum_i x[(g,i),(h,k)] B[i,j]
      The stationary operand is the DATA tile, so the contraction
      index flip (i out, k up to partitions) comes free - no
      transposes anywhere.

  MM2 (weights-stationary, batched over GRP supertiles):
      o2 = blockdiag(A,A).T @ [u_0|...|u_7]
      -> o2[(h,l), (q,g,j)] = sum_k A[k,l] u[(h,k),(q,g,j)]
      where u is o1 copied PSUM->SBUF (fp16). Split into 2 matmuls of
      N=512 (PSUM bank limit).

fp16 (1 cycle/row on TensorE) halves PE streaming cycles vs fp32/f32r,
and fp16 I/O halves HBM traffic: the kernel is DMA-bound at ~32 MB/core
(~91 us at ~360 GB/s per-core HBM). GRP=8 amortizes the per-instruction
PSUM access overhead of the DVE cast (120 cyc) and ACT gelu (172 cyc);
half-block DMA slabs (512 KB) shorten the pipeline fill/drain.

Sharding: pure data-parallel over the token dim - 2048 tokens per
core, A/B/bias replicated, no collectives. Host pre-permutes x into
[blk, half, (g,i), (s,h,k)] fp16 slabs so every DMA is one contiguous
512 KB transfer, and inverse-permutes y ([blk, half, (h,l), (s,g,j)]).
"""

import numpy as np

N_CORES = 8
TOKENS = 16384
D = 4096
TPC = TOKENS // N_CORES  # tokens per core

_CACHE = {}


def _build(tpc, mm_impl, with_bias, n_cores):
    import concourse.bacc as bacc
    import concourse.mybir as mybir
    import concourse.tile as tile

    f32 = mybir.dt.float32
    mmdt = {"f16": mybir.dt.float16, "bf16": mybir.dt.bfloat16}[mm_impl]

    nsuper = tpc // 4
    NB = min(32, nsuper)          # supertiles per block
    assert nsuper % NB == 0
    nblocks = nsuper // NB
    GRP = 8 if NB % 8 == 0 else 4  # supertiles per PSUM pack
    assert NB % GRP == 0
    NH = NB // 2                  # supertiles per half-slab

    nc = bacc.Bacc(
        "TRN2",
        target_bir_lowering=False,
        debug=False,
        num_devices=n_cores,
    )
    x_d = nc.dram_tensor(
        "x", [nblocks, 2, 128, NH * 128], mmdt, kind="ExternalInput"
    ).ap()
    b_d = nc.dram_tensor("bmat", [128, 128], mmdt, kind="ExternalInput").ap()
    a_d = nc.dram_tensor("amat", [128, 128], mmdt, kind="ExternalInput").ap()
    if with_bias:
        bias_d = nc.dram_tensor("bias_t", [128, 128], f32, kind="ExternalInput").ap()
    y_d = nc.dram_tensor(
        "y", [nblocks, 2, 128, NH * 128], mmdt, kind="ExternalOutput"
    ).ap()

    with tile.TileContext(nc) as tc:
        with (
            tc.tile_pool(name="const", bufs=1) as constp,
            tc.tile_pool(name="xp", bufs=4) as xp,
            tc.tile_pool(name="up", bufs=4) as up,
            tc.tile_pool(name="yp", bufs=4) as yp,
            tc.tile_pool(name="ps1", bufs=2, space="PSUM") as ps1,
            tc.tile_pool(name="ps2", bufs=2, space="PSUM") as ps2,
        ):
            bmat = constp.tile([128, 128], mmdt)
            nc.sync.dma_start(bmat[:], b_d)
            amat = constp.tile([128, 128], mmdt)
            nc.sync.dma_start(amat[:], a_d)
            if with_bias:
                bias_t = constp.tile([128, 128], f32)
                nc.sync.dma_start(bias_t[:], bias_d)

            for blk in range(nblocks):
                xbig = xp.tile([128, NB * 128], mmdt)
                ybig = yp.tile([128, NB * 128], mmdt)
                for half in range(2):
                    nc.sync.dma_start(
                        xbig[:, half * NH * 128 : (half + 1) * NH * 128],
                        x_d[blk, half],
                    )

                for grp in range(NB // GRP):
                    o1 = ps1.tile([128, GRP * 128], f32)
                    o2 = ps2.tile([128, GRP * 128], f32)
                    u = up.tile([128, GRP * 128], mmdt)
                    if blk == 0 and grp == 0:
                        # HAM warmup: ~4us of back-to-back matmuls on the
                        # weight tiles while the first x slab is still in
                        # flight, so real matmuls start at K=8/8 (2.4 GHz).
                        for w in range(80):
                            nc.tensor.matmul(o1[:, :128], amat[:], bmat[:])
                    for q in range(GRP):
                        s = grp * GRP + q
                        nc.tensor.matmul(
                            o1[:, q * 128 : (q + 1) * 128],
                            xbig[:, s * 128 : (s + 1) * 128],
                            bmat[:],
                        )
                    # cast PSUM->SBUF in two 1-bank slices (a 2-bank AP
                    # pays a measurable penalty on DVE)
                    nc.vector.tensor_copy(u[:, :512], o1[:, :512])
                    nc.vector.tensor_copy(u[:, 512:], o1[:, 512:])
                    for m2 in range(GRP // 4):
                        nc.tensor.matmul(
                            o2[:, m2 * 512 : (m2 + 1) * 512],
                            amat[:],
                            u[:, m2 * 512 : (m2 + 1) * 512],
                        )
                    ydst = ybig[:, grp * GRP * 128 : (grp + 1) * GRP * 128]
                    nc.scalar.activation(
                        ydst, o2[:], mybir.ActivationFunctionType.Gelu
                    )
                    if with_bias:
                        bseg = ydst.rearrange("p (q f) -> p q f", f=128)
                        bsrc = bias_t[:].unsqueeze(1).broadcast_to([128, GRP, 128])
                        nc.vector.tensor_add(bseg, bseg, bsrc)
                    # ship each half-slab as soon as its second gelu lands:
                    # shortens the pipeline drain to two grps + one 512KB DMA
                    if grp % 2 == 1:
                        half = grp // 2
                        nc.gpsimd.dma_start(
                            y_d[blk, half],
                            ybig[:, half * NH * 128 : (half + 1) * NH * 128],
                        )

    nc.compile()
    return nc


def _get_nc(tpc, mm_impl, with_bias, n_cores=N_CORES):
    key = (tpc, mm_impl, with_bias, n_cores)
    if key not in _CACHE:
        _CACHE[key] = _build(*key)
    return _CACHE[key]


def _make_weights(A, B, np_dt):
    Bd = np.zeros((128, 128), np.float32)
    Bd[:64, :64] = B
    Bd[64:, 64:] = B
    Ad = np.zeros((128, 128), np.float32)
    Ad[:64, :64] = A
    Ad[64:, 64:] = A
    return {"bmat": Bd.astype(np_dt), "amat": Ad.astype(np_dt)}


def _run(x, A, B, bias, mm_impl="f16", tpc=TPC, trace=False):
    import ml_dtypes
    from concourse.bass_utils import run_bass_kernel_spmd

    np_dt = {"f16": np.float16, "bf16": ml_dtypes.bfloat16}[mm_impl]

    n = x.shape[0]
    n_cores = n // tpc
    assert n == n_cores * tpc

    with_bias = bool(np.any(bias))
    nc = _get_nc(tpc, mm_impl, with_bias, n_cores)
    wmaps = _make_weights(np.asarray(A, np.float32), np.asarray(B, np.float32), np_dt)

    nsuper = tpc // 4
    NB = min(32, nsuper)
    nblocks = nsuper // NB
    NH = NB // 2

    def permute_x(xs):
        # token t = g*(tpc/2) + h*(tpc/4) + blk*NB + s2*NH + sl
        # feature = i*64+k  ->  [blk, s2, (g,i), (sl,h,k)]
        v = xs.reshape(2, 2, nblocks, 2, NH, 64, 64).transpose(2, 3, 0, 5, 4, 1, 6)
        return np.ascontiguousarray(
            v.reshape(nblocks, 2, 128, NH * 128).astype(np_dt)
        )

    def unpermute_y(yd):
        # [blk, s2, (h,l), (sl,g,j)] -> tokens x features (feature = j*64+l)
        v = np.asarray(yd).reshape(nblocks, 2, 2, 64, NH, 2, 64)
        v = v.transpose(5, 2, 0, 1, 4, 6, 3)
        return v.reshape(tpc, D).astype(np.float32)

    in_maps = []
    for c in range(n_cores):
        m = {"x": permute_x(np.asarray(x[c * tpc : (c + 1) * tpc], dtype=np.float32))}
        m.update(wmaps)
        if with_bias:
            bt = bias.astype(np.float32).reshape(64, 64).T  # [l, j]
            m["bias_t"] = np.ascontiguousarray(np.tile(bt, (2, 2)))
        in_maps.append(m)

    res = run_bass_kernel_spmd(
        nc, in_maps, list(range(n_cores)), trace=trace,
        trace_cores=list(range(n_cores)) if trace else None,
    )
    y = np.concatenate([unpermute_y(r["y"]) for r in res.results], axis=0)
    return y.astype(np.float32), res


def kernel(x, A, B, bias):
    y, _ = _run(
        np.asarray(x), np.asarray(A), np.asarray(B), np.asarray(bias),
        mm_impl="f16",
    )
    return y


# revision 25
# speedup vs baseline: 1.0605x; 1.0605x over previous
"""Kronecker layer forward on 8 TRN2 NeuronCores.

Computes y = gelu_exact(x @ kron(B, A)) + bias for
  x [16384, 4096] f32, A [64, 64], B [64, 64], bias [4096].

Math: with x3 = x.reshape(n, 64, 64) (feature f = i*64 + k),
  y[b, j*64+l] = sum_{i,k} x3[b,i,k] * B[i,j] * A[k,l].

Per supertile s we pick 4 tokens t(g,h) = g*(tpc/2) + h*(tpc/4) + s
(g,h in {0,1}) and form one 128x128 SBUF tile
  xt[(g,i), (h,k)] = x[t(g,h), i*64+k]
then chain two TensorE matmuls in fp16:

  MM1 (data-stationary): o1 = xt.T @ blockdiag(B,B)
      -> o1[(h,k), (g,j)] = sum_i x[(g,i),(h,k)] B[i,j]
      The stationary operand is the DATA tile, so the contraction
      index flip (i out, k up to partitions) comes free - no
      transposes anywhere.

  MM2 (weights-stationary, batched over GRP supertiles):
      o2 = blockdiag(A,A).T @ [u_0|...|u_7]
      -> o2[(h,l), (q,g,j)] = sum_k A[k,l] u[(h,k),(q,g,j)]
      where u is o1 copied PSUM->SBUF (fp16). Split into 2 matmuls of
      N=512 (PSUM bank limit).

fp16 (1 cycle/row on TensorE) halves PE streaming cycles vs fp32/f32r,
and fp16 I/O halves HBM traffic: the kernel is DMA-bound at ~32 MB/core
(~91 us at ~360 GB/s per-core HBM). GRP=8 amortizes the per-instruction
PSUM access overhead of the DVE cast (120 cyc) and ACT gelu (172 cyc);
half-block DMA slabs (512 KB) shorten the pipeline fill/drain.

Sharding: pure data-parallel over the token dim - 2048 tokens per
core, A/B/bias replicated, no collectives. Host pre-permutes x into
[blk, half, (g,i), (s,h,k)] fp16 slabs so every DMA is one contiguous
512 KB transfer, and inverse-permutes y ([blk, half, (h,l), (s,g,j)]).
"""

import numpy as np

N_CORES = 8
TOKENS = 16384
D = 4096
TPC = TOKENS // N_CORES  # tokens per core

_CACHE = {}


def _build(tpc, mm_impl, with_bias, n_cores):
    import concourse.bacc as bacc
    import concourse.mybir as mybir
    import concourse.tile as tile

    f32 = mybir.dt.float32
    mmdt = {"f16": mybir.dt.float16, "bf16": mybir.dt.bfloat16}[mm_impl]

    nsuper = tpc // 4
    NB = min(64, nsuper)          # supertiles per block
    assert nsuper % NB == 0
    nblocks = nsuper // NB
    GRP = 8 if NB % 8 == 0 else 4  # supertiles per PSUM pack
    assert NB % GRP == 0
    NH = NB // 2                  # supertiles per half-slab

    nc = bacc.Bacc(
        "TRN2",
        target_bir_lowering=False,
        debug=False,
        num_devices=n_cores,
    )
    x_d = nc.dram_tensor(
        "x", [nblocks, 2, 128, NH * 128], mmdt, kind="ExternalInput"
    ).ap()
    b_d = nc.dram_tensor("bmat", [128, 128], mmdt, kind="ExternalInput").ap()
    a_d = nc.dram_tensor("amat", [128, 128], mmdt, kind="ExternalInput").ap()
    if with_bias:
        bias_d = nc.dram_tensor("bias_t", [128, 128], f32, kind="ExternalInput").ap()
    y_d = nc.dram_tensor(
        "y", [nblocks, 2, 128, NH * 128], mmdt, kind="ExternalOutput"
    ).ap()

    with tile.TileContext(nc) as tc:
        with (
            tc.tile_pool(name="const", bufs=1) as constp,
            tc.tile_pool(name="xp", bufs=4) as xp,
            tc.tile_pool(name="up", bufs=4) as up,
            tc.tile_pool(name="yp", bufs=4) as yp,
            tc.tile_pool(name="ps1", bufs=2, space="PSUM") as ps1,
            tc.tile_pool(name="ps2", bufs=2, space="PSUM") as ps2,
        ):
            bmat = constp.tile([128, 128], mmdt)
            nc.sync.dma_start(bmat[:], b_d)
            amat = constp.tile([128, 128], mmdt)
            nc.sync.dma_start(amat[:], a_d)
            if with_bias:
                bias_t = constp.tile([128, 128], f32)
                nc.sync.dma_start(bias_t[:], bias_d)

            for blk in range(nblocks):
                xbig = xp.tile([128, NB * 128], mmdt)
                ybig = yp.tile([128, NB * 128], mmdt)
                for half in range(2):
                    nc.sync.dma_start(
                        xbig[:, half * NH * 128 : (half + 1) * NH * 128],
                        x_d[blk, half],
                    )

                for grp in range(NB // GRP):
                    o1 = ps1.tile([128, GRP * 128], f32)
                    o2 = ps2.tile([128, GRP * 128], f32)
                    u = up.tile([128, GRP * 128], mmdt)
                    if blk == 0 and grp == 0:
                        # HAM warmup: ~4us of back-to-back matmuls on the
                        # weight tiles while the first x slab is still in
                        # flight, so real matmuls start at K=8/8 (2.4 GHz).
                        for w in range(80):
                            nc.tensor.matmul(o1[:, :128], amat[:], bmat[:])
                    for q in range(GRP):
                        s = grp * GRP + q
                        nc.tensor.matmul(
                            o1[:, q * 128 : (q + 1) * 128],
                            xbig[:, s * 128 : (s + 1) * 128],
                            bmat[:],
                        )
                    # cast PSUM->SBUF in two 1-bank slices (a 2-bank AP
                    # pays a measurable penalty on DVE)
                    nc.vector.tensor_copy(u[:, :512], o1[:, :512])
                    nc.vector.tensor_copy(u[:, 512:], o1[:, 512:])
                    for m2 in range(GRP // 4):
                        nc.tensor.matmul(
                            o2[:, m2 * 512 : (m2 + 1) * 512],
                            amat[:],
                            u[:, m2 * 512 : (m2 + 1) * 512],
                        )
                    ydst = ybig[:, grp * GRP * 128 : (grp + 1) * GRP * 128]
                    nc.scalar.activation(
                        ydst, o2[:], mybir.ActivationFunctionType.Gelu
                    )
                    if with_bias:
                        bseg = ydst.rearrange("p (q f) -> p q f", f=128)
                        bsrc = bias_t[:].unsqueeze(1).broadcast_to([128, GRP, 128])
                        nc.vector.tensor_add(bseg, bseg, bsrc)
                    # ship each half-slab as soon as its last gelu lands:
                    # shortens the pipeline drain to half a block + one DMA
                    hgrp = NB // GRP // 2
                    if (grp + 1) % hgrp == 0:
                        half = (grp + 1) // hgrp - 1
                        nc.gpsimd.dma_start(
                            y_d[blk, half],
                            ybig[:, half * NH * 128 : (half + 1) * NH * 128],
                        )

    nc.compile()
    return nc


def _get_nc(tpc, mm_impl, with_bias, n_cores=N_CORES):
    key = (tpc, mm_impl, with_bias, n_cores)
    if key not in _CACHE:
        _CACHE[key] = _build(*key)
    return _CACHE[key]


def _make_weights(A, B, np_dt):
    Bd = np.zeros((128, 128), np.float32)
    Bd[:64, :64] = B
    Bd[64:, 64:] = B
    Ad = np.zeros((128, 128), np.float32)
    Ad[:64, :64] = A
    Ad[64:, 64:] = A
    return {"bmat": Bd.astype(np_dt), "amat": Ad.astype(np_dt)}


def _run(x, A, B, bias, mm_impl="f16", tpc=TPC, trace=False):
    import ml_dtypes
    from concourse.bass_utils import run_bass_kernel_spmd

    np_dt = {"f16": np.float16, "bf16": ml_dtypes.bfloat16}[mm_impl]

    n = x.shape[0]
    n_cores = n // tpc
    assert n == n_cores * tpc

    with_bias = bool(np.any(bias))
    nc = _get_nc(tpc, mm_impl, with_bias, n_cores)
    wmaps = _make_weights(np.asarray(A, np.float32), np.asarray(B, np.float32), np_dt)

    nsuper = tpc // 4
    NB = min(64, nsuper)
    nblocks = nsuper // NB
    NH = NB // 2

    def permute_x(xs):
        # token t = g*(tpc/2) + h*(tpc/4) + blk*NB + s2*NH + sl
        # feature = i*64+k  ->  [blk, s2, (g,i), (sl,h,k)]
        v = xs.reshape(2, 2, nblocks, 2, NH, 64, 64).transpose(2, 3, 0, 5, 4, 1, 6)
        return np.ascontiguousarray(
            v.reshape(nblocks, 2, 128, NH * 128).astype(np_dt)
        )

    def unpermute_y(yd):
        # [blk, s2, (h,l), (sl,g,j)] -> tokens x features (feature = j*64+l)
        v = np.asarray(yd).reshape(nblocks, 2, 2, 64, NH, 2, 64)
        v = v.transpose(5, 2, 0, 1, 4, 6, 3)
        return v.reshape(tpc, D).astype(np.float32)

    in_maps = []
    for c in range(n_cores):
        m = {"x": permute_x(np.asarray(x[c * tpc : (c + 1) * tpc], dtype=np.float32))}
        m.update(wmaps)
        if with_bias:
            bt = bias.astype(np.float32).reshape(64, 64).T  # [l, j]
            m["bias_t"] = np.ascontiguousarray(np.tile(bt, (2, 2)))
        in_maps.append(m)

    res = run_bass_kernel_spmd(
        nc, in_maps, list(range(n_cores)), trace=trace,
        trace_cores=list(range(n_cores)) if trace else None,
    )
    y = np.concatenate([unpermute_y(r["y"]) for r in res.results], axis=0)
    return y.astype(np.float32), res


def kernel(x, A, B, bias):
    y, _ = _run(
        np.asarray(x), np.asarray(A), np.asarray(B), np.asarray(bias),
        mm_impl="f16",
    )
    return y
